# revision 23
# baseline (speedup 1.0000x reference)
"""CapsuleLayer (dynamic routing) Trainium2 kernel, v4.

Problem: B=128, I=1152 input capsules (A=8), O=10 output capsules (OA=16),
3 routing iterations.  Data-parallel over batch: 8 cores x 16 examples.

Per-core layout: SBUF partition p = is*16 + b (is = i mod 8, b = local batch),
c = i // 8 in the free dim, vote coordinate n = oa*O + o (o innermost).

Design:
  - votes: 144 k=64 matmuls (stationary = per-c block-diagonal x over is,
    host-built); 5.3MB total input DMA over 16 queues.
  - delta = sum_oa votes*v on the TENSOR engine: 16 PSUM-accumulating
    identity matmuls per 48-c chunk, accumulating INTO a persistent
    logits PSUM tile across routing iterations (logits += delta for free).
  - The two votes-sized elementwise muls (tmp = votes*v, wv = route*votes)
    run on DVE only (GPSIMD shares the SBUF port with DVE: co-running it
    measured ~4x slowdown on concurrent DVE ops).
  - Repeated-stationary matmuls skip LDWEIGHTS via the InstMatmult flag.
  - s1 matmuls pipelined into the votes phase with a 2-group lag.
  - sqrt in squash via exp(0.5*ln(x)) to stay in one ACT table set.
"""

import numpy as np
import ml_dtypes

B, I, A, O, OA = 128, 1152, 8, 10, 16
NCORES = 8
BL = B // NCORES        # 16 examples per core
IS8 = 8                 # i-positions per chunk
C = I // IS8            # 144 chunks
N = O * OA              # 160, n = oa*O + o
P = 128                 # p = is*BL + b
K64 = IS8 * A           # 64 contraction rows in the votes matmul
CP = C // 2             # 72 paired chunks (k=128 votes matmuls)
N2 = 2 * N              # 320 columns per paired votes matmul
NUM_ROUTING = 3
SW = 3                  # c's per s-matmul (n = SW*N = 480)
CW = 48                 # c's per routing chunk (= one PSUM bank of logits)
NCH = C // CW           # 3 chunks

PATCH_LDW = True        # skip LDWEIGHTS on repeated-stationary matmuls

_NC_CACHE = {}


def _build_nc():
    from contextlib import ExitStack

    import concourse.tile as tile
    import concourse.mybir as mybir
    from concourse import bacc

    F32 = mybir.dt.float32
    BF16 = mybir.dt.bfloat16
    AF = mybir.ActivationFunctionType
    ALU = mybir.AluOpType
    AX = mybir.AxisListType

    def noldw(mi):
        if PATCH_LDW:
            mi.ins.ldweights = False
        return mi

    nc = bacc.Bacc()
    w2c_d = nc.dram_tensor("w2c", [P, CP, N2], BF16, kind="ExternalInput")
    xbd_d = nc.dram_tensor("xbd", [P, CP, P], BF16, kind="ExternalInput")
    bsel_d = nc.dram_tensor("bsel", [P, BL], BF16, kind="ExternalInput")
    brep_d = nc.dram_tensor("brep", [BL, P], BF16, kind="ExternalInput")
    bias_d = nc.dram_tensor("biasr", [BL, N], F32, kind="ExternalInput")
    ident_d = nc.dram_tensor("ident", [P, P], BF16, kind="ExternalInput")
    vout_d = nc.dram_tensor("vout", [BL, N], F32, kind="ExternalOutput")

    with ExitStack() as ctx:
        tc = ctx.enter_context(tile.TileContext(nc))
        st = ctx.enter_context(tc.tile_pool(name="static", bufs=1))
        itp = ctx.enter_context(tc.tile_pool(name="itp", bufs=1))

        votes = st.tile([P, C, N], BF16)
        big = st.tile([P, C, N], BF16)      # tmp (votes*v) / wv (route*votes)
        expb = st.tile([P, C, O], BF16)
        route = st.tile([P, C, O], BF16)
        z = st.tile([P, C], F32)
        rz = st.tile([P, C], F32)
        bsel = st.tile([P, BL], BF16)
        brep = st.tile([BL, P], BF16)
        biasr = st.tile([BL, N], F32)
        ident = st.tile([P, P], BF16)
        vrep = st.tile([P, N], BF16)


        pss = ctx.enter_context(tc.tile_pool(name="pss", bufs=1, space="PSUM"))
        s_ps = pss.tile([BL, SW * N], F32, tag="sps")

        # ---- votes phase ----
        with tc.tile_pool(name="ph1", bufs=1) as ph1, tc.tile_pool(
            name="psv", bufs=2, space="PSUM"
        ) as psv:
            w2c = ph1.tile([P, CP, N2], BF16)
            xbd = ph1.tile([P, CP, P], BF16)
            NDMA = 12
            dstep = CP // NDMA
            nc.scalar.dma_start(out=bsel[:], in_=bsel_d[:])
            nc.scalar.dma_start(out=brep[:], in_=brep_d[:])
            nc.scalar.dma_start(out=biasr[:], in_=bias_d[:])
            nc.scalar.dma_start(out=ident[:], in_=ident_d[:])
            for q in range(NDMA):
                sl = slice(q * dstep, (q + 1) * dstep)
                nc.sync.dma_start(out=w2c[:, sl, :], in_=w2c_d[:, sl, :])
                nc.sync.dma_start(out=xbd[:, sl, :], in_=xbd_d[:, sl, :])
            # prewarm the exp/ln ACT table set during the head DMAs
            warm = itp.tile([BL, OA], F32, tag="warm")
            nc.scalar.activation(warm[:], biasr[:, 0:OA], AF.Exp)
            nc.scalar.activation(warm[:], biasr[:, 0:OA], AF.Ln)

            GP3 = 3       # paired chunks per psum tile (one 320-col mm per bank)
            NG = CP // GP3  # 24 groups of 6 c's
            pstiles = {}
            s1_state = {"next": 0, "done": 0}

            def do_copy(g):
                ps = pstiles.pop(g)
                src = ps[:, :, 0:N2].rearrange("p b (j n) -> p b j n", j=2)
                dst = votes[:, g * 2 * GP3 : (g + 1) * 2 * GP3, :].rearrange(
                    "p (b j) n -> p b j n", b=GP3
                )
                nc.scalar.copy(dst[:, 0:2], src[:, 0:2])
                nc.vector.tensor_copy(dst[:, 2:3], src[:, 2:3])

            def do_s1(upto_c):
                # emit s1 matmuls for all complete SW-triples below upto_c
                while s1_state["next"] + SW <= upto_c:
                    c0 = s1_state["next"]
                    rhs = votes[:, c0 : c0 + SW, :].rearrange("p c n -> p (c n)")
                    mi = nc.tensor.matmul(
                        s_ps[:],
                        lhsT=bsel[:],
                        rhs=rhs,
                        start=(c0 == 0),
                        stop=(c0 + SW == C),
                    )
                    if c0 > 0:
                        noldw(mi)
                    s1_state["next"] = c0 + SW

            for g in range(NG):
                ps = psv.tile([P, GP3, 512], F32, tag="pv")
                pstiles[g] = ps
                for j in range(GP3):
                    cp = g * GP3 + j
                    nc.tensor.matmul(
                        ps[:, j, 0:N2],
                        lhsT=xbd[:, cp, :],
                        rhs=w2c[:, cp, :],
                        start=True,
                        stop=True,
                    )
                do_copy(g)
                if g >= 5:
                    do_s1((g - 4) * 2 * GP3)
            do_s1(C)

        lps = ctx.enter_context(tc.tile_pool(name="lps", bufs=1, space="PSUM"))
        logits_ps = [
            lps.tile([P, 512], F32, name=f"logits{g}", tag=f"logits{g}")
            for g in range(NCH)
        ]
        z_ps = [
            lps.tile([P, 512], F32, name=f"zps{g}", tag=f"zps{g}")
            for g in range(NCH)
        ]
        dps = ctx.enter_context(tc.tile_pool(name="dps", bufs=1, space="PSUM"))

        v4 = votes[:].rearrange("p c (oa o) -> p c oa o", o=O)
        b4 = big[:].rearrange("p c (oa o) -> p c oa o", o=O)

        def squash(t):
            """s_ps -> vt [BL, N] f32."""
            sa = itp.tile([BL, N], F32, tag="sa")
            nc.vector.reduce_sum(
                sa[:],
                s_ps[:].rearrange("b (c n) -> b n c", c=SW),
                axis=AX.X,
            )
            s_t = itp.tile([BL, N], F32, tag="stile")
            if t == 1:
                nc.vector.scalar_tensor_tensor(
                    s_t[:], sa[:], 1.0 / O, biasr[:], op0=ALU.mult, op1=ALU.add
                )
            else:
                nc.vector.tensor_add(s_t[:], sa[:], biasr[:])
            sq = itp.tile([BL, N], F32, tag="sq")
            nc.vector.tensor_mul(sq[:], s_t[:], s_t[:])
            nsq = itp.tile([BL, OA], F32, tag="nsq")
            nc.vector.reduce_sum(
                nsq[:], sq[:].rearrange("b (oa o) -> b oa o", o=O), axis=AX.X
            )
            nsq1 = itp.tile([BL, OA], F32, tag="nsq1")
            nc.vector.tensor_scalar_add(nsq1[:], nsq[:], 1.0)
            rn1 = itp.tile([BL, OA], F32, tag="rn1")
            nc.vector.reciprocal_approx_fast(rn1[:], nsq1[:])
            lnn = itp.tile([BL, OA], F32, tag="lnn")
            nc.scalar.activation(lnn[:], nsq[:], AF.Ln)
            sr = itp.tile([BL, OA], F32, tag="sr")
            nc.scalar.activation(sr[:], lnn[:], AF.Exp, scale=0.5)
            f = itp.tile([BL, OA], F32, tag="f")
            nc.vector.tensor_mul(f[:], sr[:], rn1[:])
            vt = itp.tile(
                [BL, N], F32 if t == NUM_ROUTING else BF16, tag=f"vt{t}"
            )
            nc.vector.tensor_mul(
                vt[:].rearrange("b (oa o) -> b oa o", o=O),
                s_t[:].rearrange("b (oa o) -> b oa o", o=O),
                f[:].unsqueeze(2).broadcast_to([BL, OA, O]),
            )
            return vt

        def make_vrep(vbf):
            vr_ps = dps.tile([P, 512], F32, tag="vrps")
            nc.tensor.matmul(
                vr_ps[:, 0:N], lhsT=brep[:], rhs=vbf[:], start=True, stop=True
            )
            nc.scalar.copy(vrep[:], vr_ps[:, 0:N])

        def transition(t):
            """delta_t -> logits (PSUM) -> route_{t+1} -> wv -> s for t+1."""
            vb = (
                vrep[:]
                .rearrange("p (oa o) -> p oa o", o=O)
                .unsqueeze(1)
                .broadcast_to([P, CW, OA, O])
            )

            def do_j(g):
                c0 = g * CW
                nc.vector.tensor_mul(b4[:, c0 : c0 + CW], v4[:, c0 : c0 + CW], vb)

            def do_k(g):
                c0 = g * CW
                for oa in range(OA):
                    mi = nc.tensor.matmul(
                        logits_ps[g][:, 0 : CW * O],
                        lhsT=ident[:],
                        rhs=b4[:, c0 : c0 + CW, oa, :],
                        start=(t == 1 and oa == 0),
                        stop=(oa == OA - 1),
                        skip_group_check=(t > 1),
                    )
                    if g > 0 or oa > 0:
                        noldw(mi)

            def do_exp_z(g):
                c0 = g * CW
                lview = logits_ps[g][:, 0 : CW * O].rearrange("p (c o) -> p c o", o=O)
                nc.scalar.activation(expb[:, c0 : c0 + CW], lview, AF.Exp)
                for o in range(O):
                    mi = nc.tensor.matmul(
                        z_ps[g][:, 0:CW],
                        lhsT=ident[:],
                        rhs=expb[:, c0 : c0 + CW, o],
                        start=(o == 0),
                        stop=(o == O - 1),
                        skip_group_check=(t > 1),
                    )
                    noldw(mi)

            def do_route_e(g):
                c0 = g * CW
                nc.vector.reciprocal_approx_fast(
                    rz[:, c0 : c0 + CW], z_ps[g][:, 0:CW]
                )
                nc.vector.tensor_mul(
                    route[:, c0 : c0 + CW],
                    expb[:, c0 : c0 + CW],
                    rz[:, c0 : c0 + CW].unsqueeze(2).broadcast_to([P, CW, O]),
                )
                rb = route[:, c0 : c0 + CW, :].unsqueeze(2).broadcast_to(
                    [P, CW, OA, O]
                )
                nc.vector.tensor_mul(b4[:, c0 : c0 + CW], v4[:, c0 : c0 + CW], rb)

            def do_s(g, last):
                for j in range(CW // SW):
                    cj = g * CW + j * SW
                    rhs = big[:, cj : cj + SW, :].rearrange("p c n -> p (c n)")
                    mi = nc.tensor.matmul(
                        s_ps[:],
                        lhsT=bsel[:],
                        rhs=rhs,
                        start=(g == 0 and j == 0),
                        stop=(last and j == CW // SW - 1),
                    )
                    if g > 0 or j > 0:
                        noldw(mi)

            # DVE: J0 J1 [r0 E0] J2 [r1 E1] [r2 E2]; PE: K0 z0 K1 z1 K2 z2 s0 s1 s2
            do_j(0)
            do_k(0)
            do_exp_z(0)
            do_j(1)
            do_k(1)
            do_route_e(0)
            do_exp_z(1)
            do_j(2)
            do_k(2)
            do_route_e(1)
            do_exp_z(2)
            do_route_e(2)
            for g in range(NCH):
                do_s(g, last=(g == NCH - 1))

        for t in range(1, NUM_ROUTING + 1):
            vt = squash(t)
            if t == NUM_ROUTING:
                nc.sync.dma_start(out=vout_d[:], in_=vt[:])
                break
            make_vrep(vt)
            transition(t)

    nc.compile()
    return nc


def get_nc():
    if "nc" not in _NC_CACHE:
        _NC_CACHE["nc"] = _build_nc()
    return _NC_CACHE["nc"]


def make_in_maps(x, weights, biases):
    bf = ml_dtypes.bfloat16
    x = np.asarray(x, np.float32)
    weights = np.asarray(weights, np.float32)
    biases = np.asarray(biases, np.float32)

    # w2c[(h,is,a), cp, h2*N + (oa,o)] = w[(2cp+h)*8+is, a, o*16+oa] * (h==h2)
    w5 = (
        weights.reshape(CP, 2, IS8, A, O, OA)
        .transpose(0, 1, 2, 3, 5, 4)
        .reshape(CP, 2, IS8, A, N)
    )
    w2c = np.zeros((CP, 2, IS8, A, 2, N), np.float32)
    for h in range(2):
        w2c[:, h, :, :, h, :] = w5[:, h]
    w2c = np.ascontiguousarray(w2c.reshape(CP, P, N2).transpose(1, 0, 2)).astype(bf)

    eye = np.eye(BL, dtype=np.float32)
    bsel = np.tile(eye, (IS8, 1)).astype(bf)  # bsel[p, b'] = delta(p % BL == b')
    brep = np.tile(eye, (1, IS8)).astype(bf)  # brep[b, p] = delta(b == p % BL)
    biasr = np.broadcast_to(biases.T.reshape(1, N), (BL, N)).astype(np.float32).copy()
    ident = np.eye(P, dtype=np.float32).astype(bf)

    in_maps = []
    idx = np.arange(IS8)
    for k in range(NCORES):
        xc = x[k * BL : (k + 1) * BL]  # [BL, I, A]
        xt = xc.reshape(BL, C, IS8, A).transpose(2, 1, 3, 0)  # [IS8, C, A, BL]
        xbd = np.zeros((C, IS8, A, IS8, BL), np.float32)
        xbd[:, idx, :, idx, :] = xt
        # [C=2*CP, (is,a)=64, (is',b)=128] -> pair chunks into k=128
        xbd = xbd.reshape(CP, 2 * IS8 * A, IS8 * BL).transpose(1, 0, 2).astype(bf)
        in_maps.append(
            {
                "w2c": w2c,
                "xbd": np.ascontiguousarray(xbd),
                "bsel": bsel,
                "brep": brep,
                "biasr": biasr,
                "ident": ident,
            }
        )
    return in_maps


def assemble_out(results):
    out = np.zeros((B, 1, O, OA), np.float32)
    for k in range(NCORES):
        v = np.asarray(results[k]["vout"], np.float32)  # [BL, N], n = oa*O + o
        out[k * BL : (k + 1) * BL, 0] = v.reshape(BL, OA, O).transpose(0, 2, 1)
    return out


def kernel(x, weights, biases):
    from concourse.bass_utils import run_bass_kernel_spmd

    nc = get_nc()
    in_maps = make_in_maps(x, weights, biases)
    res = run_bass_kernel_spmd(nc, in_maps, list(range(NCORES)))
    return assemble_out(res.results)


# revision 24
# speedup vs baseline: 1.0080x; 1.0080x over previous
"""CapsuleLayer (dynamic routing) Trainium2 kernel, v4.

Problem: B=128, I=1152 input capsules (A=8), O=10 output capsules (OA=16),
3 routing iterations.  Data-parallel over batch: 8 cores x 16 examples.

Per-core layout: SBUF partition p = is*16 + b (is = i mod 8, b = local batch),
c = i // 8 in the free dim, vote coordinate n = oa*O + o (o innermost).

Design:
  - votes: 144 k=64 matmuls (stationary = per-c block-diagonal x over is,
    host-built); 5.3MB total input DMA over 16 queues.
  - delta = sum_oa votes*v on the TENSOR engine: 16 PSUM-accumulating
    identity matmuls per 48-c chunk, accumulating INTO a persistent
    logits PSUM tile across routing iterations (logits += delta for free).
  - The two votes-sized elementwise muls (tmp = votes*v, wv = route*votes)
    run on DVE only (GPSIMD shares the SBUF port with DVE: co-running it
    measured ~4x slowdown on concurrent DVE ops).
  - Repeated-stationary matmuls skip LDWEIGHTS via the InstMatmult flag.
  - s1 matmuls pipelined into the votes phase with a 2-group lag.
  - sqrt in squash via exp(0.5*ln(x)) to stay in one ACT table set.
"""

import numpy as np
import ml_dtypes

B, I, A, O, OA = 128, 1152, 8, 10, 16
NCORES = 8
BL = B // NCORES        # 16 examples per core
IS8 = 8                 # i-positions per chunk
C = I // IS8            # 144 chunks
N = O * OA              # 160, n = oa*O + o
P = 128                 # p = is*BL + b
K64 = IS8 * A           # 64 contraction rows in the votes matmul
CP = C // 2             # 72 paired chunks (k=128 votes matmuls)
N2 = 2 * N              # 320 columns per paired votes matmul
NUM_ROUTING = 3
SW = 3                  # c's per s-matmul (n = SW*N = 480)
CW = 48                 # c's per routing chunk (= one PSUM bank of logits)
NCH = C // CW           # 3 chunks

PATCH_LDW = True        # skip LDWEIGHTS on repeated-stationary matmuls

_NC_CACHE = {}


def _build_nc():
    from contextlib import ExitStack

    import concourse.tile as tile
    import concourse.mybir as mybir
    from concourse import bacc

    F32 = mybir.dt.float32
    BF16 = mybir.dt.bfloat16
    AF = mybir.ActivationFunctionType
    ALU = mybir.AluOpType
    AX = mybir.AxisListType

    def noldw(mi):
        if PATCH_LDW:
            mi.ins.ldweights = False
        return mi

    nc = bacc.Bacc()
    w2c_d = nc.dram_tensor("w2c", [P, CP, N2], BF16, kind="ExternalInput")
    xbd_d = nc.dram_tensor("xbd", [P, CP, P], BF16, kind="ExternalInput")
    bsel_d = nc.dram_tensor("bsel", [P, BL], BF16, kind="ExternalInput")
    brep_d = nc.dram_tensor("brep", [BL, P], BF16, kind="ExternalInput")
    bias_d = nc.dram_tensor("biasr", [BL, N], F32, kind="ExternalInput")
    ident_d = nc.dram_tensor("ident", [P, P], BF16, kind="ExternalInput")
    vout_d = nc.dram_tensor("vout", [BL, N], F32, kind="ExternalOutput")

    with ExitStack() as ctx:
        tc = ctx.enter_context(tile.TileContext(nc))
        st = ctx.enter_context(tc.tile_pool(name="static", bufs=1))
        itp = ctx.enter_context(tc.tile_pool(name="itp", bufs=1))

        votes = st.tile([P, C, N], BF16)
        big = st.tile([P, C, N], BF16)      # tmp (votes*v) / wv (route*votes)
        expb = st.tile([P, C, O], BF16)
        route = st.tile([P, C, O], BF16)
        z = st.tile([P, C], F32)
        rz = st.tile([P, C], F32)
        bsel = st.tile([P, BL], BF16)
        brep = st.tile([BL, P], BF16)
        biasr = st.tile([BL, N], F32)
        ident = st.tile([P, P], BF16)
        vrep = st.tile([P, N], BF16)


        pss = ctx.enter_context(tc.tile_pool(name="pss", bufs=1, space="PSUM"))
        s_ps = pss.tile([BL, SW * N], F32, tag="sps")

        # ---- votes phase ----
        with tc.tile_pool(name="ph1", bufs=1) as ph1, tc.tile_pool(
            name="psv", bufs=2, space="PSUM"
        ) as psv:
            w2c = ph1.tile([P, CP, N2], BF16)
            xbd = ph1.tile([P, CP, P], BF16)
            NDMA = 12
            dstep = CP // NDMA
            nc.scalar.dma_start(out=bsel[:], in_=bsel_d[:])
            nc.scalar.dma_start(out=brep[:], in_=brep_d[:])
            nc.scalar.dma_start(out=biasr[:], in_=bias_d[:])
            nc.scalar.dma_start(out=ident[:], in_=ident_d[:])
            for q in range(NDMA):
                sl = slice(q * dstep, (q + 1) * dstep)
                nc.sync.dma_start(out=w2c[:, sl, :], in_=w2c_d[:, sl, :])
                nc.sync.dma_start(out=xbd[:, sl, :], in_=xbd_d[:, sl, :])
            # prewarm the exp/ln ACT table set during the head DMAs
            warm = itp.tile([BL, OA], F32, tag="warm")
            nc.scalar.activation(warm[:], biasr[:, 0:OA], AF.Exp)
            nc.scalar.activation(warm[:], biasr[:, 0:OA], AF.Ln)

            GP3 = 3       # paired chunks per psum tile (one 320-col mm per bank)
            NG = CP // GP3  # 24 groups of 6 c's
            pstiles = {}
            s1_state = {"next": 0, "done": 0}

            def do_copy(g):
                ps = pstiles.pop(g)
                src = ps[:, :, 0:N2].rearrange("p b (j n) -> p b j n", j=2)
                dst = votes[:, g * 2 * GP3 : (g + 1) * 2 * GP3, :].rearrange(
                    "p (b j) n -> p b j n", b=GP3
                )
                nc.scalar.copy(dst[:, 0:2], src[:, 0:2])
                nc.vector.tensor_copy(dst[:, 2:3], src[:, 2:3])

            def do_s1(upto_c):
                # emit s1 matmuls for all complete SW-triples below upto_c
                while s1_state["next"] + SW <= upto_c:
                    c0 = s1_state["next"]
                    rhs = votes[:, c0 : c0 + SW, :].rearrange("p c n -> p (c n)")
                    mi = nc.tensor.matmul(
                        s_ps[:],
                        lhsT=bsel[:],
                        rhs=rhs,
                        start=(c0 == 0),
                        stop=(c0 + SW == C),
                    )
                    if c0 > 0:
                        noldw(mi)
                    s1_state["next"] = c0 + SW

            for g in range(NG):
                ps = psv.tile([P, GP3, 512], F32, tag="pv")
                pstiles[g] = ps
                for j in range(GP3):
                    cp = g * GP3 + j
                    nc.tensor.matmul(
                        ps[:, j, 0:N2],
                        lhsT=xbd[:, cp, :],
                        rhs=w2c[:, cp, :],
                        start=True,
                        stop=True,
                    )
                do_copy(g)
                if g >= 2:
                    do_s1((g - 1) * 2 * GP3)
            do_s1(C)

        lps = ctx.enter_context(tc.tile_pool(name="lps", bufs=1, space="PSUM"))
        logits_ps = [
            lps.tile([P, 512], F32, name=f"logits{g}", tag=f"logits{g}")
            for g in range(NCH)
        ]
        z_ps = [
            lps.tile([P, 512], F32, name=f"zps{g}", tag=f"zps{g}")
            for g in range(NCH)
        ]
        dps = ctx.enter_context(tc.tile_pool(name="dps", bufs=1, space="PSUM"))

        v4 = votes[:].rearrange("p c (oa o) -> p c oa o", o=O)
        b4 = big[:].rearrange("p c (oa o) -> p c oa o", o=O)

        def squash(t):
            """s_ps -> vt [BL, N] f32."""
            sa = itp.tile([BL, N], F32, tag="sa")
            nc.vector.reduce_sum(
                sa[:],
                s_ps[:].rearrange("b (c n) -> b n c", c=SW),
                axis=AX.X,
            )
            s_t = itp.tile([BL, N], F32, tag="stile")
            if t == 1:
                nc.vector.scalar_tensor_tensor(
                    s_t[:], sa[:], 1.0 / O, biasr[:], op0=ALU.mult, op1=ALU.add
                )
            else:
                nc.vector.tensor_add(s_t[:], sa[:], biasr[:])
            sq = itp.tile([BL, N], F32, tag="sq")
            nc.vector.tensor_mul(sq[:], s_t[:], s_t[:])
            nsq = itp.tile([BL, OA], F32, tag="nsq")
            nc.vector.reduce_sum(
                nsq[:], sq[:].rearrange("b (oa o) -> b oa o", o=O), axis=AX.X
            )
            nsq1 = itp.tile([BL, OA], F32, tag="nsq1")
            nc.vector.tensor_scalar_add(nsq1[:], nsq[:], 1.0)
            rn1 = itp.tile([BL, OA], F32, tag="rn1")
            nc.vector.reciprocal_approx_fast(rn1[:], nsq1[:])
            lnn = itp.tile([BL, OA], F32, tag="lnn")
            nc.scalar.activation(lnn[:], nsq[:], AF.Ln)
            sr = itp.tile([BL, OA], F32, tag="sr")
            nc.scalar.activation(sr[:], lnn[:], AF.Exp, scale=0.5)
            f = itp.tile([BL, OA], F32, tag="f")
            nc.vector.tensor_mul(f[:], sr[:], rn1[:])
            vt = itp.tile(
                [BL, N], F32 if t == NUM_ROUTING else BF16, tag=f"vt{t}"
            )
            nc.vector.tensor_mul(
                vt[:].rearrange("b (oa o) -> b oa o", o=O),
                s_t[:].rearrange("b (oa o) -> b oa o", o=O),
                f[:].unsqueeze(2).broadcast_to([BL, OA, O]),
            )
            return vt

        def make_vrep(vbf):
            vr_ps = dps.tile([P, 512], F32, tag="vrps")
            nc.tensor.matmul(
                vr_ps[:, 0:N], lhsT=brep[:], rhs=vbf[:], start=True, stop=True
            )
            nc.scalar.copy(vrep[:], vr_ps[:, 0:N])

        def transition(t):
            """delta_t -> logits (PSUM) -> route_{t+1} -> wv -> s for t+1."""
            vb = (
                vrep[:]
                .rearrange("p (oa o) -> p oa o", o=O)
                .unsqueeze(1)
                .broadcast_to([P, CW, OA, O])
            )

            def do_j(g):
                c0 = g * CW
                nc.vector.tensor_mul(b4[:, c0 : c0 + CW], v4[:, c0 : c0 + CW], vb)

            def do_k(g):
                c0 = g * CW
                for oa in range(OA):
                    mi = nc.tensor.matmul(
                        logits_ps[g][:, 0 : CW * O],
                        lhsT=ident[:],
                        rhs=b4[:, c0 : c0 + CW, oa, :],
                        start=(t == 1 and oa == 0),
                        stop=(oa == OA - 1),
                        skip_group_check=(t > 1),
                    )
                    if g > 0 or oa > 0:
                        noldw(mi)

            def do_exp_z(g):
                c0 = g * CW
                lview = logits_ps[g][:, 0 : CW * O].rearrange("p (c o) -> p c o", o=O)
                nc.scalar.activation(expb[:, c0 : c0 + CW], lview, AF.Exp)
                for o in range(O):
                    mi = nc.tensor.matmul(
                        z_ps[g][:, 0:CW],
                        lhsT=ident[:],
                        rhs=expb[:, c0 : c0 + CW, o],
                        start=(o == 0),
                        stop=(o == O - 1),
                        skip_group_check=(t > 1),
                    )
                    noldw(mi)

            def do_route_e(g):
                c0 = g * CW
                nc.vector.reciprocal_approx_fast(
                    rz[:, c0 : c0 + CW], z_ps[g][:, 0:CW]
                )
                nc.vector.tensor_mul(
                    route[:, c0 : c0 + CW],
                    expb[:, c0 : c0 + CW],
                    rz[:, c0 : c0 + CW].unsqueeze(2).broadcast_to([P, CW, O]),
                )
                rb = route[:, c0 : c0 + CW, :].unsqueeze(2).broadcast_to(
                    [P, CW, OA, O]
                )
                nc.vector.tensor_mul(b4[:, c0 : c0 + CW], v4[:, c0 : c0 + CW], rb)

            def do_s(g, last):
                for j in range(CW // SW):
                    cj = g * CW + j * SW
                    rhs = big[:, cj : cj + SW, :].rearrange("p c n -> p (c n)")
                    mi = nc.tensor.matmul(
                        s_ps[:],
                        lhsT=bsel[:],
                        rhs=rhs,
                        start=(g == 0 and j == 0),
                        stop=(last and j == CW // SW - 1),
                    )
                    if g > 0 or j > 0:
                        noldw(mi)

            # DVE: J0 J1 [r0 E0] J2 [r1 E1] [r2 E2]; PE: K0 z0 K1 z1 K2 z2 s0 s1 s2
            do_j(0)
            do_k(0)
            do_exp_z(0)
            do_j(1)
            do_k(1)
            do_route_e(0)
            do_exp_z(1)
            do_j(2)
            do_k(2)
            do_route_e(1)
            do_exp_z(2)
            do_route_e(2)
            for g in range(NCH):
                do_s(g, last=(g == NCH - 1))

        for t in range(1, NUM_ROUTING + 1):
            vt = squash(t)
            if t == NUM_ROUTING:
                nc.sync.dma_start(out=vout_d[:], in_=vt[:])
                break
            make_vrep(vt)
            transition(t)

    nc.compile()
    return nc


def get_nc():
    if "nc" not in _NC_CACHE:
        _NC_CACHE["nc"] = _build_nc()
    return _NC_CACHE["nc"]


def make_in_maps(x, weights, biases):
    bf = ml_dtypes.bfloat16
    x = np.asarray(x, np.float32)
    weights = np.asarray(weights, np.float32)
    biases = np.asarray(biases, np.float32)

    # w2c[(h,is,a), cp, h2*N + (oa,o)] = w[(2cp+h)*8+is, a, o*16+oa] * (h==h2)
    w5 = (
        weights.reshape(CP, 2, IS8, A, O, OA)
        .transpose(0, 1, 2, 3, 5, 4)
        .reshape(CP, 2, IS8, A, N)
    )
    w2c = np.zeros((CP, 2, IS8, A, 2, N), np.float32)
    for h in range(2):
        w2c[:, h, :, :, h, :] = w5[:, h]
    w2c = np.ascontiguousarray(w2c.reshape(CP, P, N2).transpose(1, 0, 2)).astype(bf)

    eye = np.eye(BL, dtype=np.float32)
    bsel = np.tile(eye, (IS8, 1)).astype(bf)  # bsel[p, b'] = delta(p % BL == b')
    brep = np.tile(eye, (1, IS8)).astype(bf)  # brep[b, p] = delta(b == p % BL)
    biasr = np.broadcast_to(biases.T.reshape(1, N), (BL, N)).astype(np.float32).copy()
    ident = np.eye(P, dtype=np.float32).astype(bf)

    in_maps = []
    idx = np.arange(IS8)
    for k in range(NCORES):
        xc = x[k * BL : (k + 1) * BL]  # [BL, I, A]
        xt = xc.reshape(BL, C, IS8, A).transpose(2, 1, 3, 0)  # [IS8, C, A, BL]
        xbd = np.zeros((C, IS8, A, IS8, BL), np.float32)
        xbd[:, idx, :, idx, :] = xt
        # [C=2*CP, (is,a)=64, (is',b)=128] -> pair chunks into k=128
        xbd = xbd.reshape(CP, 2 * IS8 * A, IS8 * BL).transpose(1, 0, 2).astype(bf)
        in_maps.append(
            {
                "w2c": w2c,
                "xbd": np.ascontiguousarray(xbd),
                "bsel": bsel,
                "brep": brep,
                "biasr": biasr,
                "ident": ident,
            }
        )
    return in_maps


def assemble_out(results):
    out = np.zeros((B, 1, O, OA), np.float32)
    for k in range(NCORES):
        v = np.asarray(results[k]["vout"], np.float32)  # [BL, N], n = oa*O + o
        out[k * BL : (k + 1) * BL, 0] = v.reshape(BL, OA, O).transpose(0, 2, 1)
    return out


def kernel(x, weights, biases):
    from concourse.bass_utils import run_bass_kernel_spmd

    nc = get_nc()
    in_maps = make_in_maps(x, weights, biases)
    res = run_bass_kernel_spmd(nc, in_maps, list(range(NCORES)))
    return assemble_out(res.results)


# revision 25
# speedup vs baseline: 1.0194x; 1.0112x over previous
"""CapsuleLayer (dynamic routing) Trainium2 kernel, v4.

Problem: B=128, I=1152 input capsules (A=8), O=10 output capsules (OA=16),
3 routing iterations.  Data-parallel over batch: 8 cores x 16 examples.

Per-core layout: SBUF partition p = is*16 + b (is = i mod 8, b = local batch),
c = i // 8 in the free dim, vote coordinate n = oa*O + o (o innermost).

Design:
  - votes: 144 k=64 matmuls (stationary = per-c block-diagonal x over is,
    host-built); 5.3MB total input DMA over 16 queues.
  - delta = sum_oa votes*v on the TENSOR engine: 16 PSUM-accumulating
    identity matmuls per 48-c chunk, accumulating INTO a persistent
    logits PSUM tile across routing iterations (logits += delta for free).
  - The two votes-sized elementwise muls (tmp = votes*v, wv = route*votes)
    run on DVE only (GPSIMD shares the SBUF port with DVE: co-running it
    measured ~4x slowdown on concurrent DVE ops).
  - Repeated-stationary matmuls skip LDWEIGHTS via the InstMatmult flag.
  - s1 matmuls pipelined into the votes phase with a 2-group lag.
  - sqrt in squash via exp(0.5*ln(x)) to stay in one ACT table set.
"""

import numpy as np
import ml_dtypes

B, I, A, O, OA = 128, 1152, 8, 10, 16
NCORES = 8
BL = B // NCORES        # 16 examples per core
IS8 = 8                 # i-positions per chunk
C = I // IS8            # 144 chunks
N = O * OA              # 160, n = oa*O + o
P = 128                 # p = is*BL + b
K64 = IS8 * A           # 64 contraction rows in the votes matmul
CP = C // 2             # 72 paired chunks (k=128 votes matmuls)
N2 = 2 * N              # 320 columns per paired votes matmul
NUM_ROUTING = 3
SW = 3                  # c's per s-matmul (n = SW*N = 480)
CW = 48                 # c's per routing chunk (= one PSUM bank of logits)
NCH = C // CW           # 3 chunks

PATCH_LDW = True        # skip LDWEIGHTS on repeated-stationary matmuls

_NC_CACHE = {}


def _build_nc():
    from contextlib import ExitStack

    import concourse.tile as tile
    import concourse.mybir as mybir
    from concourse import bacc

    F32 = mybir.dt.float32
    BF16 = mybir.dt.bfloat16
    AF = mybir.ActivationFunctionType
    ALU = mybir.AluOpType
    AX = mybir.AxisListType

    def noldw(mi):
        if PATCH_LDW:
            mi.ins.ldweights = False
        return mi

    nc = bacc.Bacc()
    w2c_d = nc.dram_tensor("w2c", [P, CP, N2], BF16, kind="ExternalInput")
    xbd_d = nc.dram_tensor("xbd", [P, CP, P], BF16, kind="ExternalInput")
    bsel_d = nc.dram_tensor("bsel", [P, BL], BF16, kind="ExternalInput")
    brep_d = nc.dram_tensor("brep", [BL, P], BF16, kind="ExternalInput")
    bias_d = nc.dram_tensor("biasr", [BL, N], F32, kind="ExternalInput")
    ident_d = nc.dram_tensor("ident", [P, P], BF16, kind="ExternalInput")
    vout_d = nc.dram_tensor("vout", [BL, N], F32, kind="ExternalOutput")

    with ExitStack() as ctx:
        tc = ctx.enter_context(tile.TileContext(nc))
        st = ctx.enter_context(tc.tile_pool(name="static", bufs=1))
        itp = ctx.enter_context(tc.tile_pool(name="itp", bufs=1))

        votes = st.tile([P, C, N], BF16)
        big = st.tile([P, C, N], BF16)      # tmp (votes*v) / wv (route*votes)
        expb = st.tile([P, C, O], BF16)
        route = st.tile([P, C, O], BF16)
        z = st.tile([P, C], F32)
        rz = st.tile([P, C], F32)
        bsel = st.tile([P, BL], BF16)
        brep = st.tile([BL, P], BF16)
        biasr = st.tile([BL, N], F32)
        ident = st.tile([P, P], BF16)
        vrep = st.tile([P, N], BF16)


        pss = ctx.enter_context(tc.tile_pool(name="pss", bufs=1, space="PSUM"))
        s_ps = pss.tile([BL, SW * N], F32, tag="sps")

        # ---- votes phase ----
        with tc.tile_pool(name="ph1", bufs=1) as ph1, tc.tile_pool(
            name="psv", bufs=2, space="PSUM"
        ) as psv:
            w2c = ph1.tile([P, CP, N2], BF16)
            xbd = ph1.tile([P, CP, P], BF16)
            NDMA = 12
            dstep = CP // NDMA
            nc.scalar.dma_start(out=bsel[:], in_=bsel_d[:])
            nc.scalar.dma_start(out=brep[:], in_=brep_d[:])
            nc.scalar.dma_start(out=biasr[:], in_=bias_d[:])
            nc.scalar.dma_start(out=ident[:], in_=ident_d[:])
            for q in range(NDMA):
                sl = slice(q * dstep, (q + 1) * dstep)
                nc.sync.dma_start(out=w2c[:, sl, :], in_=w2c_d[:, sl, :])
                nc.sync.dma_start(out=xbd[:, sl, :], in_=xbd_d[:, sl, :])
            # prewarm the exp/ln ACT table set during the head DMAs
            warm = itp.tile([BL, OA], F32, tag="warm")
            nc.scalar.activation(warm[:], biasr[:, 0:OA], AF.Exp)
            nc.scalar.activation(warm[:], biasr[:, 0:OA], AF.Ln)

            GP3 = 3       # paired chunks per psum tile (one 320-col mm per bank)
            NG = CP // GP3  # 24 groups of 6 c's
            pstiles = {}
            s1_state = {"next": 0, "done": 0}

            def do_copy(g):
                ps = pstiles.pop(g)
                src = ps[:, :, 0:N2].rearrange("p b (j n) -> p b j n", j=2)
                dst = votes[:, g * 2 * GP3 : (g + 1) * 2 * GP3, :].rearrange(
                    "p (b j) n -> p b j n", b=GP3
                )
                nc.scalar.copy(dst[:, 0:2], src[:, 0:2])
                nc.vector.tensor_copy(dst[:, 2:3], src[:, 2:3])

            def do_s1(upto_c):
                # emit s1 matmuls for all complete SW-triples below upto_c
                while s1_state["next"] + SW <= upto_c:
                    c0 = s1_state["next"]
                    rhs = votes[:, c0 : c0 + SW, :].rearrange("p c n -> p (c n)")
                    mi = nc.tensor.matmul(
                        s_ps[:],
                        lhsT=bsel[:],
                        rhs=rhs,
                        start=(c0 == 0),
                        stop=(c0 + SW == C),
                    )
                    if c0 > 0:
                        noldw(mi)
                    s1_state["next"] = c0 + SW

            for g in range(NG):
                ps = psv.tile([P, GP3, 512], F32, tag="pv")
                pstiles[g] = ps
                for j in range(GP3):
                    cp = g * GP3 + j
                    nc.tensor.matmul(
                        ps[:, j, 0:N2],
                        lhsT=xbd[:, cp, :],
                        rhs=w2c[:, cp, :],
                        start=True,
                        stop=True,
                    )
                do_copy(g)
                if g >= 2:
                    do_s1((g - 1) * 2 * GP3)
            do_s1(C)

        lps = ctx.enter_context(tc.tile_pool(name="lps", bufs=1, space="PSUM"))
        logits_ps = [
            lps.tile([P, 512], F32, name=f"logits{g}", tag=f"logits{g}")
            for g in range(NCH)
        ]
        z_ps = [
            lps.tile([P, 512], F32, name=f"zps{g}", tag=f"zps{g}")
            for g in range(NCH)
        ]
        dps = ctx.enter_context(tc.tile_pool(name="dps", bufs=1, space="PSUM"))

        v4 = votes[:].rearrange("p c (oa o) -> p c oa o", o=O)
        b4 = big[:].rearrange("p c (oa o) -> p c oa o", o=O)

        def squash(t):
            """s_ps -> vt [BL, N] f32."""
            sa = itp.tile([BL, N], F32, tag="sa")
            nc.vector.reduce_sum(
                sa[:],
                s_ps[:].rearrange("b (c n) -> b n c", c=SW),
                axis=AX.X,
            )
            s_t = itp.tile([BL, N], F32, tag="stile")
            if t == 1:
                nc.vector.scalar_tensor_tensor(
                    s_t[:], sa[:], 1.0 / O, biasr[:], op0=ALU.mult, op1=ALU.add
                )
            else:
                nc.vector.tensor_add(s_t[:], sa[:], biasr[:])
            sq = itp.tile([BL, N], F32, tag="sq")
            nc.vector.tensor_mul(sq[:], s_t[:], s_t[:])
            nsq = itp.tile([BL, OA], F32, tag="nsq")
            nc.vector.reduce_sum(
                nsq[:], sq[:].rearrange("b (oa o) -> b oa o", o=O), axis=AX.X
            )
            nsq1 = itp.tile([BL, OA], F32, tag="nsq1")
            nc.vector.tensor_scalar_add(nsq1[:], nsq[:], 1.0)
            rn1 = itp.tile([BL, OA], F32, tag="rn1")
            nc.vector.reciprocal_approx_fast(rn1[:], nsq1[:])
            lnn = itp.tile([BL, OA], F32, tag="lnn")
            nc.scalar.activation(lnn[:], nsq[:], AF.Ln)
            sr = itp.tile([BL, OA], F32, tag="sr")
            nc.scalar.activation(sr[:], lnn[:], AF.Exp, scale=0.5)
            f = itp.tile([BL, OA], F32, tag="f")
            nc.vector.tensor_mul(f[:], sr[:], rn1[:])
            vt = itp.tile(
                [BL, N], F32 if t == NUM_ROUTING else BF16, tag=f"vt{t}"
            )
            nc.vector.tensor_mul(
                vt[:].rearrange("b (oa o) -> b oa o", o=O),
                s_t[:].rearrange("b (oa o) -> b oa o", o=O),
                f[:].unsqueeze(2).broadcast_to([BL, OA, O]),
            )
            return vt

        def make_vrep(vbf):
            vr_ps = dps.tile([P, 512], F32, tag="vrps")
            nc.tensor.matmul(
                vr_ps[:, 0:N], lhsT=brep[:], rhs=vbf[:], start=True, stop=True
            )
            nc.scalar.copy(vrep[:], vr_ps[:, 0:N])

        def transition(t):
            """delta_t -> logits (PSUM) -> route_{t+1} -> wv -> s for t+1."""
            vb = (
                vrep[:]
                .rearrange("p (oa o) -> p oa o", o=O)
                .unsqueeze(1)
                .broadcast_to([P, CW, OA, O])
            )

            def do_j(g):
                c0 = g * CW
                nc.vector.tensor_mul(b4[:, c0 : c0 + CW], v4[:, c0 : c0 + CW], vb)

            def do_k(g):
                c0 = g * CW
                for oa in range(OA):
                    mi = nc.tensor.matmul(
                        logits_ps[g][:, 0 : CW * O],
                        lhsT=ident[:],
                        rhs=b4[:, c0 : c0 + CW, oa, :],
                        start=(t == 1 and oa == 0),
                        stop=(oa == OA - 1),
                        skip_group_check=(t > 1),
                    )
                    if g > 0 or oa > 0:
                        noldw(mi)

            def do_exp_z(g):
                c0 = g * CW
                lview = logits_ps[g][:, 0 : CW * O].rearrange("p (c o) -> p c o", o=O)
                nc.scalar.activation(expb[:, c0 : c0 + CW], lview, AF.Exp)

            def do_route_e(g):
                c0 = g * CW
                nc.vector.reduce_sum(
                    z[:, c0 : c0 + CW], expb[:, c0 : c0 + CW], axis=AX.X
                )
                nc.vector.reciprocal_approx_fast(
                    rz[:, c0 : c0 + CW], z[:, c0 : c0 + CW]
                )
                nc.vector.tensor_mul(
                    route[:, c0 : c0 + CW],
                    expb[:, c0 : c0 + CW],
                    rz[:, c0 : c0 + CW].unsqueeze(2).broadcast_to([P, CW, O]),
                )
                rb = route[:, c0 : c0 + CW, :].unsqueeze(2).broadcast_to(
                    [P, CW, OA, O]
                )
                nc.vector.tensor_mul(b4[:, c0 : c0 + CW], v4[:, c0 : c0 + CW], rb)

            def do_s(g, last):
                for j in range(CW // SW):
                    cj = g * CW + j * SW
                    rhs = big[:, cj : cj + SW, :].rearrange("p c n -> p (c n)")
                    mi = nc.tensor.matmul(
                        s_ps[:],
                        lhsT=bsel[:],
                        rhs=rhs,
                        start=(g == 0 and j == 0),
                        stop=(last and j == CW // SW - 1),
                    )
                    if g > 0 or j > 0:
                        noldw(mi)

            # DVE: J0 J1 [r0 E0] J2 [r1 E1] [r2 E2]; PE: K0 z0 K1 z1 K2 z2 s0 s1 s2
            do_j(0)
            do_k(0)
            do_exp_z(0)
            do_j(1)
            do_k(1)
            do_route_e(0)
            do_exp_z(1)
            do_j(2)
            do_k(2)
            do_route_e(1)
            do_exp_z(2)
            do_route_e(2)
            for g in range(NCH):
                do_s(g, last=(g == NCH - 1))

        for t in range(1, NUM_ROUTING + 1):
            vt = squash(t)
            if t == NUM_ROUTING:
                nc.sync.dma_start(out=vout_d[:], in_=vt[:])
                break
            make_vrep(vt)
            transition(t)

    nc.compile()
    return nc


def get_nc():
    if "nc" not in _NC_CACHE:
        _NC_CACHE["nc"] = _build_nc()
    return _NC_CACHE["nc"]


def make_in_maps(x, weights, biases):
    bf = ml_dtypes.bfloat16
    x = np.asarray(x, np.float32)
    weights = np.asarray(weights, np.float32)
    biases = np.asarray(biases, np.float32)

    # w2c[(h,is,a), cp, h2*N + (oa,o)] = w[(2cp+h)*8+is, a, o*16+oa] * (h==h2)
    w5 = (
        weights.reshape(CP, 2, IS8, A, O, OA)
        .transpose(0, 1, 2, 3, 5, 4)
        .reshape(CP, 2, IS8, A, N)
    )
    w2c = np.zeros((CP, 2, IS8, A, 2, N), np.float32)
    for h in range(2):
        w2c[:, h, :, :, h, :] = w5[:, h]
    w2c = np.ascontiguousarray(w2c.reshape(CP, P, N2).transpose(1, 0, 2)).astype(bf)

    eye = np.eye(BL, dtype=np.float32)
    bsel = np.tile(eye, (IS8, 1)).astype(bf)  # bsel[p, b'] = delta(p % BL == b')
    brep = np.tile(eye, (1, IS8)).astype(bf)  # brep[b, p] = delta(b == p % BL)
    biasr = np.broadcast_to(biases.T.reshape(1, N), (BL, N)).astype(np.float32).copy()
    ident = np.eye(P, dtype=np.float32).astype(bf)

    in_maps = []
    idx = np.arange(IS8)
    for k in range(NCORES):
        xc = x[k * BL : (k + 1) * BL]  # [BL, I, A]
        xt = xc.reshape(BL, C, IS8, A).transpose(2, 1, 3, 0)  # [IS8, C, A, BL]
        xbd = np.zeros((C, IS8, A, IS8, BL), np.float32)
        xbd[:, idx, :, idx, :] = xt
        # [C=2*CP, (is,a)=64, (is',b)=128] -> pair chunks into k=128
        xbd = xbd.reshape(CP, 2 * IS8 * A, IS8 * BL).transpose(1, 0, 2).astype(bf)
        in_maps.append(
            {
                "w2c": w2c,
                "xbd": np.ascontiguousarray(xbd),
                "bsel": bsel,
                "brep": brep,
                "biasr": biasr,
                "ident": ident,
            }
        )
    return in_maps


def assemble_out(results):
    out = np.zeros((B, 1, O, OA), np.float32)
    for k in range(NCORES):
        v = np.asarray(results[k]["vout"], np.float32)  # [BL, N], n = oa*O + o
        out[k * BL : (k + 1) * BL, 0] = v.reshape(BL, OA, O).transpose(0, 2, 1)
    return out


def kernel(x, weights, biases):
    from concourse.bass_utils import run_bass_kernel_spmd

    nc = get_nc()
    in_maps = make_in_maps(x, weights, biases)
    res = run_bass_kernel_spmd(nc, in_maps, list(range(NCORES)))
    return assemble_out(res.results)


# revision 26
# speedup vs baseline: 1.0290x; 1.0095x over previous
"""CapsuleLayer (dynamic routing) Trainium2 kernel.

Problem: B=128, I=1152 input capsules (A=8), O=10 output capsules (OA=16),
3 routing iterations.  Data-parallel over batch: 8 cores x 16 examples.

Per-core layout: SBUF partition p = is*16 + b (is = i mod 8, b = local batch),
c = i // 8 in the free dim, vote coordinate n = oa*O + o (o innermost).

Design:
  - votes: 144 k=64 matmuls (stationary = per-c block-diagonal x over is,
    host-built); 5.3MB total input DMA over 16 queues.
  - delta = sum_oa votes*v on the TENSOR engine: 16 PSUM-accumulating
    identity matmuls per 48-c chunk, accumulating INTO a persistent
    logits PSUM tile across routing iterations (logits += delta for free).
  - The two votes-sized elementwise muls (tmp = votes*v, wv = route*votes)
    run on DVE only (GPSIMD shares the SBUF port with DVE: co-running it
    measured ~4x slowdown on concurrent DVE ops).
  - Repeated-stationary matmuls skip LDWEIGHTS via the InstMatmult flag.
  - s1 matmuls pipelined into the votes phase with a 2-group lag.
  - sqrt in squash via exp(0.5*ln(x)) to stay in one ACT table set.
"""

import numpy as np
import ml_dtypes

B, I, A, O, OA = 128, 1152, 8, 10, 16
NCORES = 8
BL = B // NCORES        # 16 examples per core
IS8 = 8                 # i-positions per chunk
C = I // IS8            # 144 chunks
N = O * OA              # 160, n = oa*O + o
P = 128                 # p = is*BL + b
K64 = IS8 * A           # 64 contraction rows in the votes matmul
CP = C // 2             # 72 paired chunks (k=128 votes matmuls)
N2 = 2 * N              # 320 columns per paired votes matmul
NUM_ROUTING = 3
SW = 3                  # c's per s-matmul (n = SW*N = 480)
CW = 48                 # c's per routing chunk (= one PSUM bank of logits)
NCH = C // CW           # 3 chunks

PATCH_LDW = True        # skip LDWEIGHTS on repeated-stationary matmuls

_NC_CACHE = {}


def _build_nc():
    from contextlib import ExitStack

    import concourse.tile as tile
    import concourse.mybir as mybir
    from concourse import bacc

    F32 = mybir.dt.float32
    BF16 = mybir.dt.bfloat16
    AF = mybir.ActivationFunctionType
    ALU = mybir.AluOpType
    AX = mybir.AxisListType

    def noldw(mi):
        if PATCH_LDW:
            mi.ins.ldweights = False
        return mi

    nc = bacc.Bacc()
    w2c_d = nc.dram_tensor("w2c", [P, CP, N2], BF16, kind="ExternalInput")
    xbd_d = nc.dram_tensor("xbd", [P, CP, P], BF16, kind="ExternalInput")
    bsel_d = nc.dram_tensor("bsel", [P, BL], BF16, kind="ExternalInput")
    brep_d = nc.dram_tensor("brep", [BL, P], BF16, kind="ExternalInput")
    bias_d = nc.dram_tensor("biasr", [BL, N], F32, kind="ExternalInput")
    ident_d = nc.dram_tensor("ident", [P, P], BF16, kind="ExternalInput")
    vout_d = nc.dram_tensor("vout", [BL, N], F32, kind="ExternalOutput")

    with ExitStack() as ctx:
        tc = ctx.enter_context(tile.TileContext(nc))
        st = ctx.enter_context(tc.tile_pool(name="static", bufs=1))
        itp = ctx.enter_context(tc.tile_pool(name="itp", bufs=1))

        votes = st.tile([P, C, N], BF16)
        big = st.tile([P, C, N], BF16)      # tmp (votes*v) / wv (route*votes)
        expb = st.tile([P, C, O], BF16)
        route = st.tile([P, C, O], BF16)
        z = st.tile([P, C], F32)
        rz = st.tile([P, C], F32)
        bsel = st.tile([P, BL], BF16)
        brep = st.tile([BL, P], BF16)
        biasr = st.tile([BL, N], F32)
        ident = st.tile([P, P], BF16)
        vrep = st.tile([P, N], BF16)


        pss = ctx.enter_context(tc.tile_pool(name="pss", bufs=1, space="PSUM"))
        s_ps = pss.tile([BL, SW * N], F32, tag="sps")

        # ---- votes phase ----
        with tc.tile_pool(name="ph1", bufs=1) as ph1, tc.tile_pool(
            name="psv", bufs=2, space="PSUM"
        ) as psv:
            w2c = ph1.tile([P, CP, N2], BF16)
            xbd = ph1.tile([P, CP, P], BF16)
            NDMA = 12
            dstep = CP // NDMA
            nc.scalar.dma_start(out=bsel[:], in_=bsel_d[:])
            nc.scalar.dma_start(out=brep[:], in_=brep_d[:])
            nc.scalar.dma_start(out=biasr[:], in_=bias_d[:])
            nc.scalar.dma_start(out=ident[:], in_=ident_d[:])
            for q in range(NDMA):
                sl = slice(q * dstep, (q + 1) * dstep)
                nc.sync.dma_start(out=w2c[:, sl, :], in_=w2c_d[:, sl, :])
                nc.sync.dma_start(out=xbd[:, sl, :], in_=xbd_d[:, sl, :])
            # prewarm the exp/ln ACT table set during the head DMAs
            warm = itp.tile([BL, OA], F32, tag="warm")
            nc.scalar.activation(warm[:], biasr[:, 0:OA], AF.Exp)
            nc.scalar.activation(warm[:], biasr[:, 0:OA], AF.Ln)

            GP3 = 3       # paired chunks per psum tile (one 320-col mm per bank)
            NG = CP // GP3  # 24 groups of 6 c's
            pstiles = {}
            s1_state = {"next": 0, "done": 0}

            def do_copy(g):
                ps = pstiles.pop(g)
                src = ps[:, :, 0:N2].rearrange("p b (j n) -> p b j n", j=2)
                dst = votes[:, g * 2 * GP3 : (g + 1) * 2 * GP3, :].rearrange(
                    "p (b j) n -> p b j n", b=GP3
                )
                nc.scalar.copy(dst[:, 0:2], src[:, 0:2])
                nc.vector.tensor_copy(dst[:, 2:3], src[:, 2:3])

            def do_s1(upto_c):
                # emit s1 matmuls for all complete SW-triples below upto_c
                while s1_state["next"] + SW <= upto_c:
                    c0 = s1_state["next"]
                    rhs = votes[:, c0 : c0 + SW, :].rearrange("p c n -> p (c n)")
                    mi = nc.tensor.matmul(
                        s_ps[:],
                        lhsT=bsel[:],
                        rhs=rhs,
                        start=(c0 == 0),
                        stop=(c0 + SW == C),
                    )
                    if c0 > 0:
                        noldw(mi)
                    s1_state["next"] = c0 + SW

            for g in range(NG):
                ps = psv.tile([P, GP3, 512], F32, tag="pv")
                pstiles[g] = ps
                for j in range(GP3):
                    cp = g * GP3 + j
                    nc.tensor.matmul(
                        ps[:, j, 0:N2],
                        lhsT=xbd[:, cp, :],
                        rhs=w2c[:, cp, :],
                        start=True,
                        stop=True,
                    )
                do_copy(g)
                if g >= 2:
                    do_s1((g - 1) * 2 * GP3)
            do_s1(C)

        lps = ctx.enter_context(tc.tile_pool(name="lps", bufs=1, space="PSUM"))
        logits_ps = [
            lps.tile([P, 512], F32, name=f"logits{g}", tag=f"logits{g}")
            for g in range(NCH)
        ]
        dps = ctx.enter_context(tc.tile_pool(name="dps", bufs=1, space="PSUM"))

        v4 = votes[:].rearrange("p c (oa o) -> p c oa o", o=O)
        b4 = big[:].rearrange("p c (oa o) -> p c oa o", o=O)

        def squash(t):
            """s_ps -> vt [BL, N] f32."""
            sa = itp.tile([BL, N], F32, tag="sa")
            nc.vector.reduce_sum(
                sa[:],
                s_ps[:].rearrange("b (c n) -> b n c", c=SW),
                axis=AX.X,
            )
            s_t = itp.tile([BL, N], F32, tag="stile")
            if t == 1:
                nc.vector.scalar_tensor_tensor(
                    s_t[:], sa[:], 1.0 / O, biasr[:], op0=ALU.mult, op1=ALU.add
                )
            else:
                nc.vector.tensor_add(s_t[:], sa[:], biasr[:])
            sq = itp.tile([BL, N], F32, tag="sq")
            nc.vector.tensor_mul(sq[:], s_t[:], s_t[:])
            nsq = itp.tile([BL, OA], F32, tag="nsq")
            nc.vector.reduce_sum(
                nsq[:], sq[:].rearrange("b (oa o) -> b oa o", o=O), axis=AX.X
            )
            nsq1 = itp.tile([BL, OA], F32, tag="nsq1")
            nc.vector.tensor_scalar_add(nsq1[:], nsq[:], 1.0)
            rn1 = itp.tile([BL, OA], F32, tag="rn1")
            nc.vector.reciprocal_approx_fast(rn1[:], nsq1[:])
            lnn = itp.tile([BL, OA], F32, tag="lnn")
            nc.scalar.activation(lnn[:], nsq[:], AF.Ln)
            sr = itp.tile([BL, OA], F32, tag="sr")
            nc.scalar.activation(sr[:], lnn[:], AF.Exp, scale=0.5)
            f = itp.tile([BL, OA], F32, tag="f")
            nc.vector.tensor_mul(f[:], sr[:], rn1[:])
            vt = itp.tile(
                [BL, N], F32 if t == NUM_ROUTING else BF16, tag=f"vt{t}"
            )
            nc.vector.tensor_mul(
                vt[:].rearrange("b (oa o) -> b oa o", o=O),
                s_t[:].rearrange("b (oa o) -> b oa o", o=O),
                f[:].unsqueeze(2).broadcast_to([BL, OA, O]),
            )
            return vt

        def make_vrep(vbf):
            vr_ps = dps.tile([P, 512], F32, tag="vrps")
            nc.tensor.matmul(
                vr_ps[:, 0:N], lhsT=brep[:], rhs=vbf[:], start=True, stop=True
            )
            nc.scalar.copy(vrep[:], vr_ps[:, 0:N])

        def transition(t):
            """delta_t -> logits (PSUM) -> route_{t+1} -> wv -> s for t+1."""
            vb = (
                vrep[:]
                .rearrange("p (oa o) -> p oa o", o=O)
                .unsqueeze(1)
                .broadcast_to([P, CW, OA, O])
            )

            def do_j(g):
                c0 = g * CW
                nc.vector.tensor_mul(b4[:, c0 : c0 + CW], v4[:, c0 : c0 + CW], vb)

            def do_k(g):
                c0 = g * CW
                for oa in range(OA):
                    mi = nc.tensor.matmul(
                        logits_ps[g][:, 0 : CW * O],
                        lhsT=ident[:],
                        rhs=b4[:, c0 : c0 + CW, oa, :],
                        start=(t == 1 and oa == 0),
                        stop=(oa == OA - 1),
                        skip_group_check=(t > 1),
                    )
                    if g > 0 or oa > 0:
                        noldw(mi)

            def do_exp_z(g):
                c0 = g * CW
                lview = logits_ps[g][:, 0 : CW * O].rearrange("p (c o) -> p c o", o=O)
                nc.scalar.activation(expb[:, c0 : c0 + CW], lview, AF.Exp)

            def do_route_e(g):
                c0 = g * CW
                nc.vector.reduce_sum(
                    z[:, c0 : c0 + CW], expb[:, c0 : c0 + CW], axis=AX.X
                )
                nc.vector.reciprocal_approx_fast(
                    rz[:, c0 : c0 + CW], z[:, c0 : c0 + CW]
                )
                nc.vector.tensor_mul(
                    route[:, c0 : c0 + CW],
                    expb[:, c0 : c0 + CW],
                    rz[:, c0 : c0 + CW].unsqueeze(2).broadcast_to([P, CW, O]),
                )
                rb = route[:, c0 : c0 + CW, :].unsqueeze(2).broadcast_to(
                    [P, CW, OA, O]
                )
                nc.vector.tensor_mul(b4[:, c0 : c0 + CW], v4[:, c0 : c0 + CW], rb)

            def do_s(g, last):
                for j in range(CW // SW):
                    cj = g * CW + j * SW
                    rhs = big[:, cj : cj + SW, :].rearrange("p c n -> p (c n)")
                    mi = nc.tensor.matmul(
                        s_ps[:],
                        lhsT=bsel[:],
                        rhs=rhs,
                        start=(g == 0 and j == 0),
                        stop=(last and j == CW // SW - 1),
                    )
                    if g > 0 or j > 0:
                        noldw(mi)

            # DVE: J0 J1 [r0 E0] J2 [r1 E1] [r2 E2]; PE: K0 z0 K1 z1 K2 z2 s0 s1 s2
            do_j(0)
            do_k(0)
            do_exp_z(0)
            do_j(1)
            do_k(1)
            do_route_e(0)
            do_exp_z(1)
            do_j(2)
            do_k(2)
            do_route_e(1)
            do_exp_z(2)
            do_route_e(2)
            for g in range(NCH):
                do_s(g, last=(g == NCH - 1))

        for t in range(1, NUM_ROUTING + 1):
            vt = squash(t)
            if t == NUM_ROUTING:
                nc.sync.dma_start(out=vout_d[:], in_=vt[:])
                break
            make_vrep(vt)
            transition(t)

    nc.compile()
    return nc


def get_nc():
    if "nc" not in _NC_CACHE:
        _NC_CACHE["nc"] = _build_nc()
    return _NC_CACHE["nc"]


def make_in_maps(x, weights, biases):
    bf = ml_dtypes.bfloat16
    x = np.asarray(x, np.float32)
    weights = np.asarray(weights, np.float32)
    biases = np.asarray(biases, np.float32)

    # w2c[(h,is,a), cp, h2*N + (oa,o)] = w[(2cp+h)*8+is, a, o*16+oa] * (h==h2)
    w5 = (
        weights.reshape(CP, 2, IS8, A, O, OA)
        .transpose(0, 1, 2, 3, 5, 4)
        .reshape(CP, 2, IS8, A, N)
    )
    w2c = np.zeros((CP, 2, IS8, A, 2, N), np.float32)
    for h in range(2):
        w2c[:, h, :, :, h, :] = w5[:, h]
    w2c = np.ascontiguousarray(w2c.reshape(CP, P, N2).transpose(1, 0, 2)).astype(bf)

    eye = np.eye(BL, dtype=np.float32)
    bsel = np.tile(eye, (IS8, 1)).astype(bf)  # bsel[p, b'] = delta(p % BL == b')
    brep = np.tile(eye, (1, IS8)).astype(bf)  # brep[b, p] = delta(b == p % BL)
    biasr = np.broadcast_to(biases.T.reshape(1, N), (BL, N)).astype(np.float32).copy()
    ident = np.eye(P, dtype=np.float32).astype(bf)

    in_maps = []
    idx = np.arange(IS8)
    for k in range(NCORES):
        xc = x[k * BL : (k + 1) * BL]  # [BL, I, A]
        xt = xc.reshape(BL, C, IS8, A).transpose(2, 1, 3, 0)  # [IS8, C, A, BL]
        xbd = np.zeros((C, IS8, A, IS8, BL), np.float32)
        xbd[:, idx, :, idx, :] = xt
        # [C=2*CP, (is,a)=64, (is',b)=128] -> pair chunks into k=128
        xbd = xbd.reshape(CP, 2 * IS8 * A, IS8 * BL).transpose(1, 0, 2).astype(bf)
        in_maps.append(
            {
                "w2c": w2c,
                "xbd": np.ascontiguousarray(xbd),
                "bsel": bsel,
                "brep": brep,
                "biasr": biasr,
                "ident": ident,
            }
        )
    return in_maps


def assemble_out(results):
    out = np.zeros((B, 1, O, OA), np.float32)
    for k in range(NCORES):
        v = np.asarray(results[k]["vout"], np.float32)  # [BL, N], n = oa*O + o
        out[k * BL : (k + 1) * BL, 0] = v.reshape(BL, OA, O).transpose(0, 2, 1)
    return out


def kernel(x, weights, biases):
    from concourse.bass_utils import run_bass_kernel_spmd

    nc = get_nc()
    in_maps = make_in_maps(x, weights, biases)
    res = run_bass_kernel_spmd(nc, in_maps, list(range(NCORES)))
    return assemble_out(res.results)
